# revision 41
# baseline (speedup 1.0000x reference)
"""MultiHeadAttention Trainium2 kernel (8 NeuronCores, SPMD).

Problem: B=2, L=2048, DK=DV=512, H=8, dh=64.
  Q = q @ WQ[h]; K = k @ WK[h]; V = v @ WV[h]       (per head)
  y = Q K^T / sqrt(L); z = softmax(y, axis=QUERY); out = z @ V
  concat heads on feature dim.

Sharding: 16 (b,h) pairs over 8 cores -> 2 heads (same batch) per core.

Per core (heads h0/h1), per k-tile (128 k rows):
  scores S[k(128), q] land in PSUM as [128, 512] quarters on a 3-buffer
  rotation; a dedicated 1-bank pool holds the V-projection / K-proj
  evacuation staging.
  exp is the bottleneck and is split across engines by whole heads:
    h0 -> ACT exact Exp activations with fused accum row sums (-> D_h0
          via two Pool adds of the four quarter sums);
    h1 -> DVE Schraudolph bf16-domain approx: one tensor_scalar
          round(score*EXP_A + EXP_B) -> int16 whose bit pattern read as
          bf16 is exp(score*SCALE) (~2% rel).  Whole (kt,h) blocks only,
          so the approx bias is common-mode in z = E/D and cancels.
          D_h1 comes from one 4x-mode bf16 copy-with-accum over the
          [128, 2048] E block.
  1/D[k] is folded into V rows (V evacuated raw so its PSUM slot frees
  without waiting on D; scaled from SBUF in 4x mode).
  AV uses E-tile slices as the stationary operand: out[q(128), ev]
  accumulates over k-tiles in a single 4-bank PSUM region, 64 moving
  rows per matmul (half the streamed rows of an out^T[ev, q] layout).
  NOTE: matmul start=True zeroes the whole PSUM bank, so only the first
  matmul per bank carries it.
Warmup interleaves Q-projection chunks with the first k-tile's scores
so the in-order PE queue never parks on a not-yet-landed DMA, and a
one-matmul prewarm starts the PE p-state ramp at t~0.  Output: bf16
[128, 16, 128] (partition-major, two overlapped DMAs); host converts
to f32 and reshapes into the full [B, L, H*dh] result.
"""

import math

import numpy as np

B = 2
L = 2048
DK = 512
H = 8
DH = 64
P = 128
NKT = L // P  # 16 k-tiles
NDC = DK // P  # 4 d-chunks
N_CORES = 8

SCALE = 1.0 / math.sqrt(float(L))
# Schraudolph in bf16-bits domain: round(raw_score*EXP_A + EXP_B) as int16,
# bitcast bf16 ~= exp(raw_score*SCALE).  C calibrated against np.exp.
EXP_A = 128.0 * math.log2(math.e) * SCALE
EXP_B = 16256.0 - 12.0

# kt values whose h1 exp runs on DVE (Schraudolph) instead of ACT.  Whole
# (kt, h) blocks only: approximating all q of a k-row keeps the Schraudolph
# bias common-mode in z = E/D so it cancels.
DVE_KTS = set(range(NKT))

# scheduling knobs (tuned via sweep)
VPROJ_MID = False  # vproj emitted between h0 and h1 scores (else after AV)
SCR_BUFS = 2
E_BUFS = 6
VF_BUFS = 4
VS_BUFS = 4
KPROJ_KTS = (0, 1, 2)
VF_DVE = False
KPC_DVE = False
TAIL4 = False
QPC_ACT = (0, 1, 2, 3)
TAIL_SWAP = True

_CACHE = {}


def _build_program():
    import concourse.bass as bass
    import concourse.tile as tile
    from concourse import bacc, mybir
    from concourse.bass import ts

    f32 = mybir.dt.float32
    bf16 = mybir.dt.bfloat16
    i16 = mybir.dt.int16
    AF = mybir.ActivationFunctionType
    ALU = mybir.AluOpType

    nc = bacc.Bacc("TRN2", target_bir_lowering=False, debug=False)

    qt_d = nc.dram_tensor("qt", [DK, L], bf16, kind="ExternalInput")
    kt_d = nc.dram_tensor("kt", [DK, L], bf16, kind="ExternalInput")
    vt_d = nc.dram_tensor("vt", [DK, L], bf16, kind="ExternalInput")
    wq_d = nc.dram_tensor("wq", [DK, P], bf16, kind="ExternalInput")
    wk_d = nc.dram_tensor("wk", [DK, P], bf16, kind="ExternalInput")
    wv_d = nc.dram_tensor("wv", [DK, P], bf16, kind="ExternalInput")
    out_d = nc.dram_tensor("out", [P, NKT, P], bf16, kind="ExternalOutput")

    with tile.TileContext(nc) as tc:
        with (
            tc.tile_pool(name="consts", bufs=1) as consts,
            tc.tile_pool(name="xin", bufs=1) as xin,
            tc.tile_pool(name="proj", bufs=1) as proj,
            tc.tile_pool(name="epool", bufs=E_BUFS) as epool,
            tc.tile_pool(name="scrpool", bufs=SCR_BUFS) as scrpool,
            tc.tile_pool(name="vfpool", bufs=VF_BUFS) as vfpool,
            tc.tile_pool(name="vspool", bufs=VS_BUFS) as vspool,
            tc.tile_pool(name="stats", bufs=1) as stats,
            tc.tile_pool(name="outp", bufs=1) as outp,
            tc.tile_pool(name="spsum", bufs=3, space="PSUM") as spsum,
            tc.tile_pool(name="vpsum", bufs=1, space="PSUM") as vpsum,
            tc.tile_pool(name="avpsum", bufs=1, space="PSUM") as avpsum,
        ):
            wq_s = consts.tile([P, NDC, P], bf16)
            wk_s = consts.tile([P, NDC, P], bf16)
            wv_s = consts.tile([P, NDC, P], bf16)
            qt_s = xin.tile([P, NDC, L], bf16)
            kt_s = xin.tile([P, NDC, L], bf16)
            vt_s = xin.tile([P, NDC, L], bf16)
            qt_r = qt_d.rearrange("(o p) l -> p o l", p=P)
            kt_r = kt_d.rearrange("(o p) l -> p o l", p=P)
            vt_r = vt_d.rearrange("(o p) l -> p o l", p=P)

            def load_chunk(sb, rr, c):
                nc.sync.dma_start(sb[:, :, ts(c, 512)], rr[:, :, ts(c, 512)])

            # PE p-state: one tiny dummy matmul at t~0 starts the 3us ramp
            # clock early (pe_busy_start doesn't reset on idle), so the real
            # warmup matmuls run at full speed
            zt = consts.tile([P, P], bf16)
            nc.gpsimd.memset(zt[:], 0.0)
            pwp = spsum.tile([P, 512], f32, tag="sco", name="prewarm")
            nc.tensor.matmul(pwp[:, 0:P], lhsT=zt[:], rhs=zt[:], start=True, stop=True)

            # critical-path-first load order
            nc.sync.dma_start(wq_s[:], wq_d.rearrange("(o p) e -> p o e", p=P))
            load_chunk(qt_s, qt_r, 0)
            nc.sync.dma_start(wk_s[:], wk_d.rearrange("(o p) e -> p o e", p=P))
            load_chunk(kt_s, kt_r, 0)
            load_chunk(qt_s, qt_r, 1)
            load_chunk(qt_s, qt_r, 2)
            load_chunk(qt_s, qt_r, 3)
            nc.sync.dma_start(wv_s[:], wv_d.rearrange("(o p) e -> p o e", p=P))
            load_chunk(vt_s, vt_r, 0)
            load_chunk(kt_s, kt_r, 1)
            load_chunk(kt_s, kt_r, 2)
            load_chunk(vt_s, vt_r, 1)
            load_chunk(kt_s, kt_r, 3)
            load_chunk(vt_s, vt_r, 2)
            load_chunk(vt_s, vt_r, 3)

            QT = proj.tile([P, L], bf16)
            KT = proj.tile([P, L], bf16)

            # [P, kt*2 + h] layout: 2-D slices (required by tensor_scalar
            # accum_out) and per-kt head pairs stay adjacent for reciprocal
            Dsum4 = stats.tile([P, NKT, 4], f32)
            Dpair = stats.tile([P, NKT, 2], f32)
            Dtot = stats.tile([P, NKT * 2], f32)
            Drec = stats.tile([P, NKT * 2], f32)

            # AV accumulator: out[q(128), qt(16), ev-pack(128)] f32 = 4
            # banks, split into two tiles so the tail's first-half evac
            # can overlap the second half's final AV matmuls
            avpA = avpsum.tile([P, 8, P], f32, tag="ava")
            avpB = avpsum.tile([P, 8, P], f32, tag="avb")

            def q_proj_chunk(qc):
                ps = spsum.tile([P, 512], f32, tag="sco", name="qproj")
                for dc in range(NDC):
                    nc.tensor.matmul(
                        ps[:],
                        lhsT=wq_s[:, dc, :],
                        rhs=qt_s[:, dc, ts(qc, 512)],
                        start=(dc == 0),
                        stop=(dc == NDC - 1),
                    )
                if qc in QPC_ACT:
                    nc.scalar.copy(QT[:, ts(qc, 512)], ps[:])
                else:
                    nc.vector.tensor_copy(QT[:, ts(qc, 512)], ps[:])

            def k_proj_chunk(c):
                ps = vpsum.tile([P, 512], f32, tag="vp", name="kproj")
                for dc in range(NDC):
                    nc.tensor.matmul(
                        ps[:],
                        lhsT=wk_s[:, dc, :],
                        rhs=kt_s[:, dc, ts(c, 512)],
                        start=(dc == 0),
                        stop=(dc == NDC - 1),
                    )
                if KPC_DVE:
                    nc.vector.tensor_copy(KT[:, ts(c, 512)], ps[:])
                else:
                    # split the evacuation across both engines
                    nc.scalar.copy(KT[:, c * 512 : c * 512 + 256], ps[:, 0:256])
                    nc.vector.tensor_copy(KT[:, c * 512 + 256 : (c + 1) * 512], ps[:, 256:512])

            # warmup: QT chunks 0/1 + KT chunk 0; QT 2/3 interleave into
            # kt=0's scores so the in-order PE queue never blocks on a
            # not-yet-landed DMA chunk
            q_proj_chunk(0)
            k_proj_chunk(0)
            q_proj_chunk(1)

            Etiles = {}
            Vstiles = {}

            def scores_exp(kt, h, qc, E):
                # one [128, 512] quarter: scores matmul + exp
                hp = h * DH
                ps = spsum.tile([P, 512], f32, tag="sco", name="sco")
                nc.tensor.matmul(
                    ps[:],
                    lhsT=KT[hp : hp + DH, ts(kt, P)],
                    rhs=QT[hp : hp + DH, ts(qc, 512)],
                    start=True,
                    stop=True,
                )
                if h == 1 and kt in DVE_KTS:
                    e16 = E[:, ts(qc, 512)].bitcast(i16)
                    nc.vector.tensor_scalar(
                        e16, ps[:], EXP_A, EXP_B, ALU.mult, ALU.add
                    )
                else:
                    nc.scalar.activation(
                        E[:, ts(qc, 512)],
                        ps[:],
                        AF.Exp,
                        scale=SCALE,
                        accum_out=Dsum4[:, kt : kt + 1, qc : qc + 1],
                    )

            def evac_half_a():
                oa = outp.tile([P, 8, P], bf16, tag="oca")
                nc.scalar.copy(oa[:], avpA[:])
                nc.scalar.dma_start(out_d[:, 0:8, :], oa[:])

            def evac_half_b():
                ob = outp.tile([P, 8, P], bf16, tag="ocb")
                nc.vector.tensor_copy(ob[:], avpB[:])
                nc.sync.dma_start(out_d[:, 8:16, :], ob[:])

            def av_block(kt, tail=False):
                E0, E1 = Etiles[kt]
                Vs = Vstiles[kt]
                for qt in range(NKT):
                    av = avpA if qt < 8 else avpB
                    qi = qt % 8
                    for h in range(2):
                        E = E0 if h == 0 else E1
                        # start=True zeroes the whole PSUM bank (4 q-tiles),
                        # so only the first matmul per bank may carry it
                        nc.tensor.matmul(
                            av[:, qi, ts(h, DH)],
                            lhsT=E[:, ts(qt, P)],
                            rhs=Vs[:, ts(h, DH)],
                            start=(kt == 0 and h == 0 and qt % 4 == 0),
                            stop=(kt == NKT - 1),
                            skip_group_check=True,
                        )
                    if tail and qt == 7:
                        # first half closed: evacuate + store while the
                        # second half's matmuls still run
                        evac_half_a()


            def vproj_vf(kt):
                # V projection; evacuate raw V immediately (no D
                # dependency) so the psum slot frees fast
                psv = vpsum.tile([P, 512], f32, tag="vp", name="psv")
                for dc in range(NDC):
                    nc.tensor.matmul(
                        psv[:, 0:P],
                        lhsT=vt_s[:, dc, ts(kt, P)],
                        rhs=wv_s[:, dc, :],
                        start=(dc == 0),
                        stop=(dc == NDC - 1),
                    )
                Vf = vfpool.tile([P, P], bf16, tag="vf")
                if VF_DVE:
                    nc.vector.tensor_copy(Vf[:], psv[:, 0:P])
                else:
                    nc.scalar.copy(Vf[:], psv[:, 0:P])
                return Vf

            def d_vs_block(kt, Vf):
                # D: h0 from the ACT exp accums (Pool sums the 4 slices);
                # h1 from one 4x-mode bf16 copy-with-accum over its E block
                nc.gpsimd.tensor_add(
                    Dpair[:, kt, 0:2],
                    Dsum4[:, kt, 0:2],
                    Dsum4[:, kt, 2:4],
                )
                nc.gpsimd.tensor_add(
                    Dtot[:, 2 * kt : 2 * kt + 1],
                    Dpair[:, kt, 0:1],
                    Dpair[:, kt, 1:2],
                )
                scr = scrpool.tile([P, L], bf16, tag="scr")
                nc.vector.tensor_scalar(
                    scr[:],
                    Etiles[kt][1][:],
                    1.0,
                    0.0,
                    ALU.mult,
                    ALU.add,
                    accum_out=Dtot[:, 2 * kt + 1 : 2 * kt + 2],
                )
                nc.vector.reciprocal(
                    Drec[:, 2 * kt : 2 * kt + 2], Dtot[:, 2 * kt : 2 * kt + 2]
                )
                Vs = vspool.tile([P, P], bf16, tag="vs")
                Vstiles[kt] = Vs
                for h in range(2):
                    # 4x-mode bf16 SBUF scale on DVE: V~ = V * (1/D)
                    nc.vector.tensor_scalar_mul(
                        Vs[:, ts(h, DH)],
                        Vf[:, ts(h, DH)],
                        Drec[:, 2 * kt + h : 2 * kt + h + 1],
                    )

            def alloc_E(kt):
                E0 = epool.tile([P, L], bf16, tag="E", name=f"E{kt}h0")
                E1 = epool.tile([P, L], bf16, tag="E", name=f"E{kt}h1")
                Etiles[kt] = (E0, E1)

            for kt in range(NKT):
                if kt == 0:
                    # interleave remaining q-projections with the first
                    # scores so each PE instruction waits only on its own
                    # DMA chunk
                    alloc_E(0)
                    E0, E1 = Etiles[0]
                    scores_exp(0, 0, 0, E0)
                    scores_exp(0, 1, 0, E1)
                    scores_exp(0, 0, 1, E0)
                    scores_exp(0, 1, 1, E1)
                    q_proj_chunk(2)
                    q_proj_chunk(3)
                    scores_exp(0, 0, 2, E0)
                    scores_exp(0, 1, 2, E1)
                    scores_exp(0, 0, 3, E0)
                    scores_exp(0, 1, 3, E1)
                    Vf = vproj_vf(0)
                    if 0 in KPROJ_KTS:
                        k_proj_chunk(1)
                    d_vs_block(0, Vf)
                else:
                    alloc_E(kt)
                    E0, E1 = Etiles[kt]
                    # interleave heads so consecutive psum-buf drains
                    # alternate between ACT and DVE
                    for qc in range(4):
                        scores_exp(kt, 0, qc, E0)
                        scores_exp(kt, 1, qc, E1)
                    Vf = vproj_vf(kt)
                    av_block(kt - 1)
                    if kt in KPROJ_KTS:
                        k_proj_chunk(KPROJ_KTS.index(kt) + 1)
                    d_vs_block(kt, Vf)

            av_block(NKT - 1, tail=True)
            evac_half_b()

    nc.compile()
    return nc


def _get_program():
    if "nc" not in _CACHE:
        _CACHE["nc"] = _build_program()
    return _CACHE["nc"]


def kernel(keys, queries, values, WQ, WK, WV):
    import ml_dtypes

    from concourse import bass_utils

    bf = ml_dtypes.bfloat16
    keys = np.asarray(keys)
    queries = np.asarray(queries)
    values = np.asarray(values)
    WQ = np.asarray(WQ)
    WK = np.asarray(WK)
    WV = np.asarray(WV)

    nc = _get_program()

    in_maps = []
    for c in range(N_CORES):
        b = c // 4
        h0 = 2 * (c % 4)
        h1 = h0 + 1
        in_maps.append(
            {
                "qt": np.ascontiguousarray(queries[b].T).astype(bf),
                "kt": np.ascontiguousarray(keys[b].T).astype(bf),
                "vt": np.ascontiguousarray(values[b].T).astype(bf),
                "wq": np.concatenate([WQ[h0], WQ[h1]], axis=1).astype(bf),
                "wk": np.concatenate([WK[h0], WK[h1]], axis=1).astype(bf),
                "wv": np.concatenate([WV[h0], WV[h1]], axis=1).astype(bf),
            }
        )

    res = bass_utils.run_bass_kernel_spmd(nc, in_maps, core_ids=list(range(N_CORES)))

    out = np.empty((B, L, H * DH), dtype=np.float32)
    for c in range(N_CORES):
        b = c // 4
        h0 = 2 * (c % 4)
        ot = np.asarray(res.results[c]["out"], dtype=np.float32)  # [128,16,128]
        out[b, :, h0 * DH : (h0 + 2) * DH] = ot.transpose(1, 0, 2).reshape(L, P)
    return out
